# revision 35
# baseline (speedup 1.0000x reference)
"""Trainium2 Bass kernel for windowed mean-pooling (segment_reduce).

Computes, for each (batch b, window w):
    out[b, w, :] = mean over t in [begins[b,w], ends'[b,w]) of features[b, t, :]
where ends' = clip(ends, begins, begins + 8) (the reference gathers at most
MAX_WINDOW=8 tokens) and empty windows produce 0 (count clamped to >= 1).

Strategy (data-parallel over batch, one sample per NeuronCore). The kernel is
bound by DMA queue time (each descriptor line costs ~87 ns + bytes/43 GB/s on
one of 16 queues), so the design minimizes shipped bytes and line count:
  - MAX-OFFSET TOKEN PACKING: for each block of 128 consecutive (sorted)
    windows, the host packs the distinct tokens that block touches (~230..
    265) back-to-back at block offsets O_i = sum_j<i max_over_cores(n_j) --
    NOT rounded to whole 128-token K-tiles. One SPMD program serves all 8
    cores; a core with fewer tokens leaves zero-padded slots. 32 K-tiles
    total (6.3 MB fp16) instead of 39 per-block-padded or 62 dense, for
    ~47 K-tile matmul passes.
  - ALL masks are host-built 0/1 fp8, one [128, 128] tile per (block,
    swept K-tile), carrying slot OWNERSHIP (a token duplicated into a
    neighbor block's slots is masked out), which a device-side
    begins/ends compare could not express. The PE consumes fp8 stationary
    against fp16 moving directly at full rate.
  - features fp16 [P, NTILES, D] in chunks (2,2,4,4,4,8,8) -> 3-12 KB
    per-partition lines, all on the ACT HWDGE ring in K order: the 16
    shared DMA queues drain lines in enqueue order, so a second ring's
    lines would interleave with and delay the feature tail.
  - out_block = S^T @ F accumulated in PSUM (512+256 col split), scaled by
    1/count (evacuations split ACT/DVE; the last block uses both engines
    in parallel), written fp16 to [P, NBLK, D] DRAM in groups (8,4,3,1)
    on the SP ring; the host un-shuffles + upcasts.
  - warm-up matmuls at t=0 ramp the PE p-state (0.65 -> 2.4 GHz needs ~3 us
    of continuous execution) while the DMA rings spin up (~3 us).
"""

import os
import sys

import numpy as np

for _p in ("/opt/trn_rl_repo", "/root/.axon_site/_ro/trn_rl_repo"):
    if os.path.isdir(_p) and _p not in sys.path:
        sys.path.insert(0, _p)

import ml_dtypes  # noqa: E402

from concourse import bacc, mybir  # noqa: E402
import concourse.tile as tile  # noqa: E402
from concourse.bass_utils import run_bass_kernel_spmd  # noqa: E402

B, T, D, W = 8, 4096, 768, 2048
MAXWIN = 8
P = 128
NBLK = W // P  # 16 window blocks of 128 windows
NWARM = 5  # PE warm-up matmuls ([P, 512] each)
F32 = mybir.dt.float32
FP16 = mybir.dt.float16
FP8 = mybir.dt.float8e4

FP8NP = ml_dtypes.float8_e4m3


def _fchunks(ntiles):
    """Feature DMA chunk sizes (K-tiles): small first for an early PE start,
    then 4-tile chunks, 8-tile at the back."""
    sizes = [2, 2, 4, 4, 4]
    left = ntiles - sum(sizes)
    while left > 8:
        sizes.append(8)
        left -= 8
    if left:
        sizes.append(left)
    return sizes


def _layout(maxn):
    """Shared slab layout from per-block max token counts: block offsets,
    per-block swept K-tile ranges, and mask-head column offsets."""
    o = np.concatenate([[0], np.cumsum(maxn)]).astype(int)
    ntiles = (int(o[-1]) + P - 1) // P
    klo = [int(o[i]) // P for i in range(NBLK)]
    khi = [(int(o[i + 1]) - 1) // P + 1 for i in range(NBLK)]
    moff = {}
    off = 0
    for i in range(NBLK):
        for k in range(klo[i], khi[i]):
            moff[(i, k)] = off
            off += P
    return o, ntiles, klo, khi, moff, off


def _build_program(maxn):
    """Build the SPMD Bass program given per-block max token counts."""
    nc = bacc.Bacc(None)
    o, ntiles, klo, khi, moff, mw8 = _layout(maxn)

    fhi_d = nc.declare_dram_parameter("fhi", [P, ntiles, D], FP16, isOutput=False)
    m8_d = nc.declare_dram_parameter("mask8", [P, mw8], FP8, isOutput=False)
    iv_d = nc.declare_dram_parameter("iv", [P, 16], F32, isOutput=False)
    out_d = nc.declare_dram_parameter("out", [P, NBLK, D], FP16, isOutput=True)

    fhi_r = fhi_d[:]
    out_r = out_d[:]

    with tile.TileContext(nc) as tc:
        with (
            tc.tile_pool(name="warmp", bufs=1) as warm_pool,
            tc.tile_pool(name="metap", bufs=1) as meta_pool,
            tc.tile_pool(name="fslab", bufs=1) as f_pool,
            tc.tile_pool(name="outp", bufs=3) as out_pool,
            tc.tile_pool(name="psum", bufs=4, space="PSUM") as psum_pool,
        ):
            # --- PE warm-up: ramp the tensor engine p-state while DMAs start.
            warm_sb = warm_pool.tile([P, 512], FP16)
            nc.vector.memset(warm_sb[:], 0.0)
            for j in range(NWARM):
                wp = psum_pool.tile([P, 512], F32, name=f"warm{j}", tag="ps")
                nc.tensor.matmul(
                    wp[:], warm_sb[:, 0:P], warm_sb[:], start=True, stop=True
                )

            # --- masks + 1/count on the SP ring (blocks 0-1's tiles first).
            # one DMA: a split would double the per-partition line count
            # (each DMA emits 128 descriptor lines at ~87 ns each).
            m8_sb = meta_pool.tile([P, mw8], FP8)
            nc.sync.dma_start(out=m8_sb[:], in_=m8_d[:])
            iv_sb = meta_pool.tile([P, 16], F32)
            nc.sync.dma_start(out=iv_sb[:], in_=iv_d[:])

            # --- feature slab chunks (fp16), ALL on the ACT ring in K order.
            fhi_tiles = []
            k2chunk = []
            k0 = 0
            for j, sz in enumerate(_fchunks(ntiles)):
                fh = f_pool.tile([P, sz, D], FP16, name=f"fh{j}", tag=f"fh{j}")
                nc.scalar.dma_start(out=fh[:], in_=fhi_r[:, k0 : k0 + sz, :])
                fhi_tiles.append(fh)
                for s in range(sz):
                    k2chunk.append((j, s))
                k0 += sz
            assert k0 == ntiles

            # --- block matmuls + evacuation + grouped output DMA.
            ogroups = (12, 3, 1)
            og_starts = []
            o0 = 0
            for g in ogroups:
                og_starts.append(o0)
                o0 += g
            assert o0 == NBLK

            gi = 0
            os_tile = None
            for i in range(NBLK):
                if i == og_starts[gi]:
                    os_tile = out_pool.tile(
                        [P, ogroups[gi], D], FP16, name=f"os{gi}", tag="os"
                    )
                ps = psum_pool.tile([P, D], F32, name=f"ps{i}", tag="ps")
                for k in range(klo[i], khi[i]):
                    mo = moff[(i, k)]
                    lh = m8_sb[:, mo : mo + P]
                    cj, cs = k2chunk[k]
                    rh = fhi_tiles[cj][:, cs, :]
                    first = k == klo[i]
                    last = k == khi[i] - 1
                    for n0, nn in ((0, 512), (512, 256)):
                        nc.tensor.matmul(
                            ps[:, n0 : n0 + nn], lh, rh[:, n0 : n0 + nn],
                            start=first, stop=(last and n0 == 512),
                        )
                oslot = i - og_starts[gi]
                if i == NBLK - 1:
                    # last block: halve the tail by evacuating on both engines
                    nc.scalar.mul(
                        out=os_tile[:, oslot, 0:512], in_=ps[:, 0:512],
                        mul=iv_sb[:, i : i + 1],
                    )
                    nc.vector.tensor_scalar(
                        os_tile[:, oslot, 512:D], ps[:, 512:D],
                        iv_sb[:, i : i + 1], None, mybir.AluOpType.mult,
                    )
                elif i % 4 != 3:
                    nc.scalar.mul(
                        out=os_tile[:, oslot, :], in_=ps[:],
                        mul=iv_sb[:, i : i + 1],
                    )
                else:
                    nc.vector.tensor_scalar(
                        os_tile[:, oslot, :], ps[:],
                        iv_sb[:, i : i + 1], None, mybir.AluOpType.mult,
                    )
                if i == og_starts[gi] + ogroups[gi] - 1:
                    # outputs on the SP ring (idle after the masks).
                    nc.sync.dma_start(
                        out=out_r[:, og_starts[gi] : i + 1, :], in_=os_tile[:]
                    )
                    gi += 1

    nc.finalize()
    return nc


def _prepare(features, begins, ends):
    feats = np.asarray(features, dtype=np.float32)
    assert feats.shape == (B, T, D), feats.shape
    b = np.clip(np.asarray(begins).astype(np.int64), 0, T - 1)
    e = np.asarray(ends).astype(np.int64)
    # Reference gathers at most MAXWIN tokens starting at b; empty -> count 1.
    e_eff = np.clip(e, b, np.minimum(b + MAXWIN, T))
    counts = np.maximum(e_eff - b, 1).astype(np.float32)
    inv = (1.0 / counts).astype(np.float32)

    # distinct tokens per (core, block); shared offsets use the max count.
    toks = {}
    maxn = np.zeros(NBLK, int)
    for c in range(B):
        for i in range(NBLK):
            ws = slice(i * P, (i + 1) * P)
            m = np.zeros(T, bool)
            for bb, ee in zip(b[c, ws], e_eff[c, ws]):
                m[bb:ee] = True
            u = np.flatnonzero(m)
            toks[(c, i)] = u
            maxn[i] = max(maxn[i], len(u))
    o, ntiles, klo, khi, moff, mw8 = _layout(maxn)

    f16 = feats.astype(np.float16)
    fhi = np.zeros((B, P, ntiles, D), np.float16)
    mask8 = np.zeros((B, P, mw8), dtype=FP8NP)
    for c in range(B):
        for i in range(NBLK):
            u = toks[(c, i)]
            n = len(u)
            oi = int(o[i])
            # scatter tokens into slab slots [oi, oi+n): slot s -> (s%P, s//P)
            sl = np.arange(oi, oi + n)
            fhi[c, sl % P, sl // P, :] = f16[c, u, :]
            # ownership-aware 0/1 masks per swept K-tile
            wlo = i * P
            bb = b[c, wlo : wlo + P]
            ee = e_eff[c, wlo : wlo + P]
            for k in range(klo[i], khi[i]):
                s0 = k * P  # tile k holds slots [s0, s0+P) on partitions 0..P
                svec = np.arange(s0, s0 + P)
                own = (svec >= oi) & (svec < oi + n)
                tk = np.zeros(P, np.int64)
                tk[own] = u[svec[own] - oi]
                m = own[:, None] & (bb[None, :] <= tk[:, None]) & (
                    tk[:, None] < ee[None, :]
                )
                mo = moff[(i, k)]
                mask8[c, :, mo : mo + P] = m.astype(FP8NP)

    in_maps = []
    for c in range(B):
        iv = np.ascontiguousarray(inv[c].reshape(NBLK, P).T)  # [P, 16]
        in_maps.append({"fhi": fhi[c], "mask8": mask8[c], "iv": iv})
    return list(maxn), in_maps


def run(features, begins, ends, trace=False):
    """Build + run on 8 NeuronCores; returns (output, BassKernelResults)."""
    maxn, in_maps = _prepare(features, begins, ends)
    nc = _build_program(maxn)
    res = run_bass_kernel_spmd(nc, in_maps, list(range(B)), trace=trace)
    # out is [P, NBLK, D] fp16 with window w = i*128 + p at [p, i, :]
    out = np.stack(
        [
            np.ascontiguousarray(
                res.results[c]["out"].astype(np.float32).transpose(1, 0, 2)
            ).reshape(W, D)
            for c in range(B)
        ],
        axis=0,
    )
    return out, res


def kernel(features, begins, ends):
    out, _ = run(features, begins, ends, trace=False)
    return out
